# revision 13
# baseline (speedup 1.0000x reference)
"""IrrepLinear (per-l grouped linear over 25 spherical-harmonic channels) on 8 trn2 cores.

Math (per batch row b):
  out[b, o*25+m] = sum_i x[b, i*25+m] * w[l(m), o, i]   (+ bias[o] for m==0)
with l(m) = the degree whose (2l+1) block contains m; IN_F=OUT_F=512, M=25, B=4096.

Sharding: data-parallel over batch (512 rows/core); weight+bias replicated.

Per-core kernel:
  - x arrives [b, i*25+m]; the contraction dim i must sit on SBUF partitions for
    the PE, so each [128b x 128i] block (strided m) is PE-transposed (f32r, 1.5
    cyc/row) into xT [i, b] via PSUM, copied back to SBUF by DVE/ACT (alternating).
  - matmul in float32r (full PE rate at N=512, fp22 mantissa): for each m,
    psum[b, o] += xT(m,ic).T @ wT(l,ic) over 4 i-chunks.
  - PSUM -> SBUF copies with free-dim stride 25 assemble the o*25+m interleave
    into two o-half staging tiles; contiguous 3.3 MB DMAs per half store out.
  - DMA queue split: SP HWDGE carries x loads, ACT HWDGE carries weights + y
    stores, so neither stream head-blocks the other.
"""

import numpy as np

import concourse.bass as bass  # noqa: F401  (bass types used via tile/bacc)
import concourse.mybir as mybir
import concourse.tile as tile
from concourse import bacc, bass_utils
from concourse.masks import make_identity

P = 128
M = 25          # (LMAX+1)^2
NF = 512        # IN_F == OUT_F
NL = 5          # LMAX+1
B = 4096
NCORES = 8
BL = B // NCORES          # 512 batch rows per core
NBC = BL // P             # 4 b-chunks per core
NIC = NF // P             # 4 i-chunks
OH = NF // 2              # o-half size (256)
IDX = [l for l in range(NL) for _ in range(2 * l + 1)]  # l(m), len 25

_cache = {}


def enable_hw_profiling():
    """Install the antenv.axon_hooks NTFF shim so trace=True yields HW timings.

    The agent image's antenv package lacks axon_hooks; reconstruct it with the
    ctypes-based hook from trn_agent_boot against /opt/axon/libaxon_pjrt.so.
    No-op if already importable or the .so is missing.
    """
    import os
    import sys
    import types

    try:
        import antenv.axon_hooks  # noqa: F401

        return True
    except ImportError:
        pass
    so_path = "/opt/axon/libaxon_pjrt.so"
    boot_dir = "/root/.axon_site"
    if not os.path.exists(so_path) or not os.path.isdir(boot_dir):
        return False
    if boot_dir not in sys.path:
        sys.path.insert(0, boot_dir)
    try:
        from trn_agent_boot.trn_boot import _ntff_profile_via_ctypes
    except ImportError:
        return False
    hook = _ntff_profile_via_ctypes(so_path)
    mod = types.ModuleType("antenv.axon_hooks")
    state = {"hook": hook}
    mod.set_axon_ntff_profile_hook = lambda h: state.update(hook=h)
    mod.get_axon_ntff_profile_hook = lambda: state["hook"]
    sys.modules["antenv.axon_hooks"] = mod
    import antenv

    antenv.axon_hooks = mod
    return hook is not None


def _build():
    f32 = mybir.dt.float32
    f32r = mybir.dt.float32r

    nc = bacc.Bacc("TRN2", debug=False, num_devices=NCORES)
    x_d = nc.dram_tensor("x", [BL, NF * M], f32r, kind="ExternalInput")
    w_d = nc.dram_tensor("wT", [NL, NF, NF], f32r, kind="ExternalInput")  # [l, i, o]
    b_d = nc.dram_tensor("biasb", [P, NF], f32, kind="ExternalInput")
    y_d = nc.dram_tensor("y", [BL, NF * M], f32, kind="ExternalOutput")

    with tile.TileContext(nc) as tc:
        with (
            tc.tile_pool(name="const", bufs=1) as cpool,
            tc.tile_pool(name="xin", bufs=2) as xpool,
            tc.tile_pool(name="xt", bufs=1) as xtpool,
            tc.tile_pool(name="out", bufs=6) as opool,
            tc.tile_pool(name="pst", bufs=3, space="PSUM") as pst,
            tc.tile_pool(name="psm", bufs=5, space="PSUM") as psm,
        ):
            ident32 = cpool.tile([P, P], f32)
            make_identity(nc, ident32)
            ident = cpool.tile([P, P], f32r)
            nc.vector.tensor_copy(ident[:], ident32[:])
            ident_r = ident[:]
            bias_sb = cpool.tile([P, NF], f32)
            nc.gpsimd.dma_start(bias_sb[:], b_d.ap())
            # all 20 [128i x 512o] weight tiles, laid out (l, ic)-major
            # (loaded on the otherwise-idle gpsimd SWDGE queue so they don't
            # head-block x loads on SP or copies/stores on ACT)
            wt = cpool.tile([P, NL * NIC * NF], f32r)
            for l in range(NL):
                for ic in range(NIC):
                    nc.gpsimd.dma_start(
                        wt[:, (l * NIC + ic) * NF : (l * NIC + ic + 1) * NF],
                        w_d.ap()[l, ic * P : (ic + 1) * P, :],
                    )

            cp_cnt = [0]

            def do_copy(dst, src):
                cp_cnt[0] += 1
                if cp_cnt[0] % 2 == 0:
                    nc.vector.tensor_copy(dst, src)
                else:
                    nc.scalar.copy(dst, src)

            for bc in range(NBC):
                # transpose phase: x [128b, i*25+m] -> xT [(ic,m)-major [128i,128b] blocks]
                xT = xtpool.tile([P, NIC * M * P], f32r)
                for ic in range(NIC):
                    xs = xpool.tile([P, P * M], f32r)
                    nc.sync.dma_start(
                        xs[:], x_d.ap()[bc * P : (bc + 1) * P, ic * P * M : (ic + 1) * P * M]
                    )
                    xs3 = xs[:].rearrange("p (i m) -> p i m", m=M)
                    for g0 in range(0, M, 4):
                        g = min(4, M - g0)
                        pt = pst.tile([P, 4 * P], f32r)
                        for j in range(g):
                            nc.tensor.transpose(
                                pt[:, j * P : (j + 1) * P], xs3[:, :, g0 + j], ident_r
                            )
                        do_copy(
                            xT[:, (ic * M + g0) * P : (ic * M + g0 + g) * P],
                            pt[:, : g * P],
                        )

                # matmul phase: per m accumulate 4 i-chunks, scatter-copy into
                # o-quarter staging tiles ([128, 3200] each, o-range of 128)
                quarters = []
                for q in range(4):
                    h = opool.tile([P, P * M], f32)
                    quarters.append((h, h[:].rearrange("p (o m) -> p o m", m=M)))
                for m in range(M):
                    l = IDX[m]
                    pm = psm.tile([P, NF], f32)
                    for ic in range(NIC):
                        nc.tensor.matmul(
                            pm[:],
                            xT[:, (ic * M + m) * P : (ic * M + m + 1) * P],
                            wt[:, (l * NIC + ic) * NF : (l * NIC + ic + 1) * NF],
                            start=(ic == 0),
                            stop=(ic == NIC - 1),
                        )
                    for q in range(4):
                        dst = quarters[q][1][:, :, m]
                        src = pm[:, q * P : (q + 1) * P]
                        if m == 0:
                            nc.vector.tensor_add(dst, src, bias_sb[:, q * P : (q + 1) * P])
                        else:
                            do_copy(dst, src)
                for q in range(4):
                    nc.scalar.dma_start(
                        y_d.ap()[bc * P : (bc + 1) * P, q * P * M : (q + 1) * P * M],
                        quarters[q][0][:],
                    )

    nc.compile()
    return nc


def _get_nc():
    if "nc" not in _cache:
        _cache["nc"] = _build()
    return _cache["nc"]


def run(input, weight, bias, trace=False):
    x = np.ascontiguousarray(np.asarray(input, dtype=np.float32).reshape(B, NF * M))
    wT = np.ascontiguousarray(
        np.asarray(weight, dtype=np.float32).transpose(0, 2, 1)
    )  # [l, i, o]
    bias_b = np.ascontiguousarray(
        np.broadcast_to(np.asarray(bias, dtype=np.float32).reshape(1, NF), (P, NF))
    )
    nc = _get_nc()
    in_maps = [
        {"x": x[c * BL : (c + 1) * BL], "wT": wT, "biasb": bias_b}
        for c in range(NCORES)
    ]
    res = bass_utils.run_bass_kernel_spmd(
        nc, in_maps, core_ids=list(range(NCORES)), trace=trace
    )
    y = np.concatenate([res.results[c]["y"] for c in range(NCORES)], axis=0)
    return y.reshape(B, NF * M, 1), res


def kernel(input, weight, bias):
    y, _ = run(input, weight, bias)
    return y


# revision 15
# speedup vs baseline: 1.1532x; 1.1532x over previous
"""IrrepLinear (per-l grouped linear over 25 spherical-harmonic channels) on 8 trn2 cores.

Math (per batch row b):
  out[b, o*25+m] = sum_i x[b, i*25+m] * w[l(m), o, i]   (+ bias[o] for m==0)
with l(m) = the degree whose (2l+1) block contains m; IN_F=OUT_F=512, M=25, B=4096.

Sharding: data-parallel over batch (512 rows/core); weight+bias replicated.

Per-core kernel:
  - x arrives [b, i*25+m]; the contraction dim i must sit on SBUF partitions for
    the PE, so each [128b x 128i] block (strided m) is PE-transposed (f32r, 1.5
    cyc/row) into xT [i, b] via PSUM, copied back to SBUF by DVE/ACT (alternating).
  - matmul in float32r (full PE rate at N=512, fp22 mantissa): for each m,
    psum[b, o] += xT(m,ic).T @ wT(l,ic) over 4 i-chunks.
  - PSUM -> SBUF copies with free-dim stride 25 assemble the o*25+m interleave
    into two o-half staging tiles; contiguous 3.3 MB DMAs per half store out.
  - DMA queue split: SP HWDGE carries x loads, ACT HWDGE carries weights + y
    stores, so neither stream head-blocks the other.
"""

import numpy as np

import concourse.bass as bass  # noqa: F401  (bass types used via tile/bacc)
import concourse.mybir as mybir
import concourse.tile as tile
from concourse import bacc, bass_utils
from concourse.masks import make_identity

P = 128
M = 25          # (LMAX+1)^2
NF = 512        # IN_F == OUT_F
NL = 5          # LMAX+1
B = 4096
NCORES = 8
BL = B // NCORES          # 512 batch rows per core
NBC = BL // P             # 4 b-chunks per core
NIC = NF // P             # 4 i-chunks
OH = NF // 2              # o-half size (256)
IDX = [l for l in range(NL) for _ in range(2 * l + 1)]  # l(m), len 25

_cache = {}


def enable_hw_profiling():
    """Install the antenv.axon_hooks NTFF shim so trace=True yields HW timings.

    The agent image's antenv package lacks axon_hooks; reconstruct it with the
    ctypes-based hook from trn_agent_boot against /opt/axon/libaxon_pjrt.so.
    No-op if already importable or the .so is missing.
    """
    import os
    import sys
    import types

    try:
        import antenv.axon_hooks  # noqa: F401

        return True
    except ImportError:
        pass
    so_path = "/opt/axon/libaxon_pjrt.so"
    boot_dir = "/root/.axon_site"
    if not os.path.exists(so_path) or not os.path.isdir(boot_dir):
        return False
    if boot_dir not in sys.path:
        sys.path.insert(0, boot_dir)
    try:
        from trn_agent_boot.trn_boot import _ntff_profile_via_ctypes
    except ImportError:
        return False
    hook = _ntff_profile_via_ctypes(so_path)
    mod = types.ModuleType("antenv.axon_hooks")
    state = {"hook": hook}
    mod.set_axon_ntff_profile_hook = lambda h: state.update(hook=h)
    mod.get_axon_ntff_profile_hook = lambda: state["hook"]
    sys.modules["antenv.axon_hooks"] = mod
    import antenv

    antenv.axon_hooks = mod
    return hook is not None


def _build():
    f32 = mybir.dt.float32
    f32r = mybir.dt.float32r

    nc = bacc.Bacc("TRN2", debug=False, num_devices=NCORES)
    x_d = nc.dram_tensor("x", [BL, NF * M], f32r, kind="ExternalInput")
    w_d = nc.dram_tensor("wT", [NL, NF, NF], f32r, kind="ExternalInput")  # [l, i, o]
    b_d = nc.dram_tensor("biasb", [P, NF], f32, kind="ExternalInput")
    y_d = nc.dram_tensor("y", [BL, NF * M], f32, kind="ExternalOutput")

    with tile.TileContext(nc) as tc:
        with (
            tc.tile_pool(name="const", bufs=1) as cpool,
            tc.tile_pool(name="xin", bufs=4) as xpool,
            tc.tile_pool(name="xt", bufs=1) as xtpool,
            tc.tile_pool(name="out", bufs=5) as opool,
            tc.tile_pool(name="pst", bufs=3, space="PSUM") as pst,
            tc.tile_pool(name="psm", bufs=5, space="PSUM") as psm,
        ):
            ident32 = cpool.tile([P, P], f32)
            make_identity(nc, ident32)
            ident = cpool.tile([P, P], f32r)
            nc.vector.tensor_copy(ident[:], ident32[:])
            ident_r = ident[:]
            bias_sb = cpool.tile([P, NF], f32)
            nc.gpsimd.dma_start(bias_sb[:], b_d.ap())
            # all 20 [128i x 512o] weight tiles, laid out (l, ic)-major
            # (loaded on the otherwise-idle gpsimd SWDGE queue so they don't
            # head-block x loads on SP or copies/stores on ACT)
            wt = cpool.tile([P, NL * NIC * NF], f32r)
            for l in range(NL):
                for ic in range(NIC):
                    nc.gpsimd.dma_start(
                        wt[:, (l * NIC + ic) * NF : (l * NIC + ic + 1) * NF],
                        w_d.ap()[l, ic * P : (ic + 1) * P, :],
                    )

            cp_cnt = [0]

            def do_copy(dst, src):
                cp_cnt[0] += 1
                if cp_cnt[0] % 2 == 0:
                    nc.vector.tensor_copy(dst, src)
                else:
                    nc.scalar.copy(dst, src)

            for bc in range(NBC):
                # transpose phase: x [128b, i*25+m] -> xT [(ic,m)-major [128i,128b] blocks]
                xT = xtpool.tile([P, NIC * M * P], f32r)
                for ic in range(NIC):
                    xs = xpool.tile([P, P * M], f32r)
                    nc.sync.dma_start(
                        xs[:], x_d.ap()[bc * P : (bc + 1) * P, ic * P * M : (ic + 1) * P * M]
                    )
                    xs3 = xs[:].rearrange("p (i m) -> p i m", m=M)
                    for g0 in range(0, M, 4):
                        g = min(4, M - g0)
                        pt = pst.tile([P, 4 * P], f32r)
                        for j in range(g):
                            nc.tensor.transpose(
                                pt[:, j * P : (j + 1) * P], xs3[:, :, g0 + j], ident_r
                            )
                        do_copy(
                            xT[:, (ic * M + g0) * P : (ic * M + g0 + g) * P],
                            pt[:, : g * P],
                        )

                # matmul phase: accumulate 4 i-chunks per (m, o-range), scatter-copy
                # into o-quarter staging tiles ([128, 3200] each, o-range of 128).
                # The last b-chunk runs o-half-outer (N=256 matmuls) so its first
                # two quarter-stores overlap the second half's compute, shrinking
                # the kernel tail.
                def emit_quarter_store(q, h, eng):
                    eng.dma_start(
                        y_d.ap()[bc * P : (bc + 1) * P, q * P * M : (q + 1) * P * M],
                        h[:],
                    )

                last = bc == NBC - 1
                oh_splits = [(0, NF)] if not last else [(0, OH), (OH, NF)]
                for o0, o1 in oh_splits:
                    quarters = []
                    for q in range(o0 // P, o1 // P):
                        h = opool.tile([P, P * M], f32)
                        quarters.append((q, h, h[:].rearrange("p (o m) -> p o m", m=M)))
                    for m in range(M):
                        l = IDX[m]
                        pm = psm.tile([P, o1 - o0], f32)
                        for ic in range(NIC):
                            nc.tensor.matmul(
                                pm[:],
                                xT[:, (ic * M + m) * P : (ic * M + m + 1) * P],
                                wt[:, (l * NIC + ic) * NF + o0 : (l * NIC + ic) * NF + o1],
                                start=(ic == 0),
                                stop=(ic == NIC - 1),
                            )
                        for qi, (q, h, h3) in enumerate(quarters):
                            dst = h3[:, :, m]
                            src = pm[:, qi * P : (qi + 1) * P]
                            if m == 0:
                                nc.vector.tensor_add(dst, src, bias_sb[:, q * P : (q + 1) * P])
                            else:
                                do_copy(dst, src)
                    for qi, (q, h, h3) in enumerate(quarters):
                        eng = nc.sync if (last and qi % 2 == 1) else nc.scalar
                        emit_quarter_store(q, h, eng)

    nc.compile()
    return nc


def _get_nc():
    if "nc" not in _cache:
        _cache["nc"] = _build()
    return _cache["nc"]


def run(input, weight, bias, trace=False):
    x = np.ascontiguousarray(np.asarray(input, dtype=np.float32).reshape(B, NF * M))
    wT = np.ascontiguousarray(
        np.asarray(weight, dtype=np.float32).transpose(0, 2, 1)
    )  # [l, i, o]
    bias_b = np.ascontiguousarray(
        np.broadcast_to(np.asarray(bias, dtype=np.float32).reshape(1, NF), (P, NF))
    )
    nc = _get_nc()
    in_maps = [
        {"x": x[c * BL : (c + 1) * BL], "wT": wT, "biasb": bias_b}
        for c in range(NCORES)
    ]
    res = bass_utils.run_bass_kernel_spmd(
        nc, in_maps, core_ids=list(range(NCORES)), trace=trace
    )
    y = np.concatenate([res.results[c]["y"] for c in range(NCORES)], axis=0)
    return y.reshape(B, NF * M, 1), res


def kernel(input, weight, bias):
    y, _ = run(input, weight, bias)
    return y
